# revision 33
# baseline (speedup 1.0000x reference)
"""HSTU block kernel for 8 Trainium2 NeuronCores — head-sharded attention.

Sharding: core c owns heads {2c, 2c+1} (a 128-feature slice of each of the
q/k/v/u projections) for ALL 4096 tokens (both batches). f1 computes only
those W1 columns, so attention is fully local per core — no kv exchange.
After gating (av*u, feature-major) one 8-core AllToAll (1 MB) re-shards
features -> tokens; LN + f2 then run token-parallel on 512 tokens per core.

All matmuls run in bf16 (full PE rate, half the DMA/SBUF of fp32). The host
pre-transposes x and pre-slices W1/b1 so the device does no input transposes
and only ~20 fat DMAs. Scores use K=64 contraction with the two heads of a
pair packed on partitions via tile_position; AV accumulates both heads into
one PSUM tile via output tile_position. The silu(scores)/S scaling is folded
into LayerNorm via eps' = S^2 * eps (LN is scale-invariant except for eps).
"""

import sys

sys.path.insert(0, "/opt/trn_rl_repo")

import ml_dtypes
import numpy as np

import concourse.bass as bass
import concourse.mybir as mybir
import concourse.tile as tile
from concourse import bacc
from concourse.bass_utils import run_bass_kernel_spmd
from concourse.masks import make_identity

F32 = mybir.dt.float32
F32R = mybir.dt.float32r
BF16 = mybir.dt.bfloat16
SILU = mybir.ActivationFunctionType.Silu
SQRT = mybir.ActivationFunctionType.Sqrt
MULT = mybir.AluOpType.mult
ADD = mybir.AluOpType.add
SUB = mybir.AluOpType.subtract

B, S, D = 2, 2048, 1024
TT = B * S         # 4096 tokens total (f1/attention scope per core)
TO = 512           # output tokens per core
KC = D // 128      # 8 feature chunks of the model dim
NTC = TT // 128    # 32 token chunks
EPS_EFF = float(S) * float(S) * 1e-5

_CACHE = {}


def _build():
    nc = bacc.Bacc(None, target_bir_lowering=False, num_devices=8)

    xT = nc.dram_tensor("xT", [D, TT], BF16, kind="ExternalInput")
    w1q = nc.dram_tensor("w1q", [D, 128], BF16, kind="ExternalInput")
    w1k = nc.dram_tensor("w1k", [D, 128], BF16, kind="ExternalInput")
    w1v = nc.dram_tensor("w1v", [D, 128], BF16, kind="ExternalInput")
    w1u = nc.dram_tensor("w1u", [D, 128], BF16, kind="ExternalInput")
    b1q = nc.dram_tensor("b1q", [128, 1], F32, kind="ExternalInput")
    b1k = nc.dram_tensor("b1k", [128, 1], F32, kind="ExternalInput")
    b1v = nc.dram_tensor("b1v", [128, 1], F32, kind="ExternalInput")
    b1u = nc.dram_tensor("b1u", [128, 1], F32, kind="ExternalInput")
    w2 = nc.dram_tensor("w2", [D, D], BF16, kind="ExternalInput")   # gamma-scaled
    b2p_in = nc.dram_tensor("b2p_bc", [128, D], F32, kind="ExternalInput")
    c2_in = nc.dram_tensor("c2_bc", [128, D], F32, kind="ExternalInput")
    y_s = nc.dram_tensor("y_s", [TO, D], F32, kind="ExternalOutput")

    xT_v = xT.rearrange("(kc p) t -> p kc t", p=128)
    w1q_v = w1q.rearrange("(kc p) f -> p kc f", p=128)
    w1k_v = w1k.rearrange("(kc p) f -> p kc f", p=128)
    w1v_v = w1v.rearrange("(kc p) f -> p kc f", p=128)
    w1u_v = w1u.rearrange("(kc p) f -> p kc f", p=128)
    w2_v = w2.rearrange("(kc p) n -> p kc n", p=128)

    with tile.TileContext(nc) as tc:
        with (
            tc.tile_pool(name="persist", bufs=1) as sbp,
            tc.tile_pool(name="small", bufs=2) as sbs,
            tc.tile_pool(name="dram", bufs=1, space="DRAM") as dram,
        ):
            # ---- constants / small params
            ident = sbp.tile([128, 128], BF16)
            make_identity(nc, ident[:])
            # stats lhsT pre-scaled by 1/D so the matmul yields means directly
            ones_col = sbp.tile([128, 1], BF16)
            nc.vector.memset(ones_col[:], 1.0 / D)
            one_f = sbp.tile([1, 1], F32)
            nc.vector.memset(one_f[:], 1.0)

            b1q_s = sbp.tile([128, 1], F32)
            b1k_s = sbp.tile([128, 1], F32)
            b1v_s = sbp.tile([128, 1], F32)
            b1u_s = sbp.tile([128, 1], F32)
            b2p_s = sbp.tile([128, D], F32)
            c2_s = sbp.tile([128, D], F32)
            eps_t = sbp.tile([128, 1], F32)
            nc.vector.memset(eps_t[:], EPS_EFF)
            nc.sync.dma_start(b1q_s[:], b1q[:, :])
            nc.sync.dma_start(b1k_s[:], b1k[:, :])
            nc.sync.dma_start(b1v_s[:], b1v[:, :])
            nc.sync.dma_start(b1u_s[:], b1u[:, :])

            # ---- persistent activations (bf16)
            kT = sbp.tile([128, TT], BF16)
            qT = sbp.tile([128, TT], BF16)
            uT = sbp.tile([128, TT], BF16)
            vTok = sbp.tile([128, NTC, 128], BF16)   # v token-major
            gatedT = sbp.tile([128, TT], BF16)
            gfull = sbp.tile([128, KC, TO], BF16)

            # block-diagonal head-pair packs: every score/AV matmul gets a
            # full 128x128 stationary tile (64-wide tiles run at half rate).
            # kblk2 block c: cols 0:64 = head-A k for kv [128c,+64) on rows
            # 0:64; cols 64:128 = head-B k for kv [128c+64,+128) on rows
            # 64:128. kblk3 is the mirror (B first half / A second half).
            kblk2 = sbp.tile([128, NTC, 128], BF16)
            kblk3 = sbp.tile([128, NTC, 128], BF16)
            vblk1 = sbp.tile([128, NTC, 128], BF16)
            vblk2 = sbp.tile([128, NTC, 128], BF16)

            w1q_t = sbp.tile([128, KC, 128], BF16)
            w1k_t = sbp.tile([128, KC, 128], BF16)
            w1v_t = sbp.tile([128, KC, 128], BF16)
            w1u_t = sbp.tile([128, KC, 128], BF16)
            w2_t = sbp.tile([128, KC, D], BF16)

            a2a_in = dram.tile([8, 128, TO], BF16, name="a2a_in")
            a2a_out = dram.tile([8, 128, TO], BF16, name="a2a_out")

            # ================= stage 1: f1 (k, q, v, u) =================
            with (
                tc.tile_pool(name="xpool", bufs=1) as xp,
                tc.tile_pool(name="ps_f1", bufs=2, space="PSUM") as ps_f1,
                tc.tile_pool(name="ps_tr", bufs=4, space="PSUM") as ps_tr,
            ):
                xTs = xp.tile([128, KC, TT], BF16)   # 64 KB/partition
                # weights interleaved with fine-grained x slices so the first
                # k-projection matmuls start as early as possible
                nc.sync.dma_start(w1k_t[:], w1k_v[:, :, :])
                nc.sync.dma_start(xTs[:, :, 0:512], xT_v[:, :, 0:512])
                nc.sync.dma_start(xTs[:, :, 512:1024], xT_v[:, :, 512:1024])
                nc.sync.dma_start(w1q_t[:], w1q_v[:, :, :])
                nc.sync.dma_start(w1v_t[:], w1v_v[:, :, :])
                nc.sync.dma_start(w1u_t[:], w1u_v[:, :, :])
                for h8 in range(2, 8):
                    nc.sync.dma_start(xTs[:, :, h8 * 512:(h8 + 1) * 512],
                                      xT_v[:, :, h8 * 512:(h8 + 1) * 512])

                nc.vector.memset(kblk2[:], 0.0)
                nc.vector.memset(kblk3[:], 0.0)
                nc.vector.memset(vblk1[:], 0.0)
                nc.vector.memset(vblk2[:], 0.0)
                # feature-major projections: out [128 f, 1024 t] per quarter
                for wt, bt, dst in ((w1k_t, b1k_s, kT), (w1q_t, b1q_s, qT),
                                    (w1v_t, b1v_s, None), (w1u_t, b1u_s, uT)):
                    for qh in range(4):
                        ps = ps_f1.tile([128, 1024], F32, tag="f1")
                        for kc in range(KC):
                            for n2 in range(2):
                                nc.tensor.matmul(
                                    ps[:, n2 * 512:(n2 + 1) * 512], wt[:, kc, :],
                                    xTs[:, kc, qh * 1024 + n2 * 512:qh * 1024 + (n2 + 1) * 512],
                                    start=(kc == 0), stop=(kc == KC - 1))
                        if dst is not None:
                            nc.scalar.activation(
                                dst[:, qh * 1024:(qh + 1) * 1024], ps[:],
                                SILU, bias=bt[:, 0:1], scale=1.0)
                        else:
                            # v: silu into a staging tile, then PE-transpose
                            vf = sbs.tile([128, 1024], BF16, tag="vf")
                            nc.scalar.activation(vf[:], ps[:], SILU,
                                                 bias=bt[:, 0:1], scale=1.0)
                            for t8 in range(8):
                                pt = ps_tr.tile([128, 128], BF16, tag="tr")
                                nc.tensor.transpose(
                                    pt[:], vf[:, t8 * 128:(t8 + 1) * 128],
                                    ident[:])
                                nc.vector.tensor_copy(
                                    vTok[:, qh * 8 + t8, :], pt[:])

                # build the block-diagonal packs (strided bulk copies)
                kTv = kT[:].rearrange("p (c t) -> p c t", t=128)
                nc.vector.tensor_copy(kblk2[0:64, :, 0:64], kTv[0:64, :, 0:64])
                nc.vector.tensor_copy(kblk2[64:128, :, 64:128],
                                      kTv[64:128, :, 64:128])
                nc.vector.tensor_copy(kblk3[0:64, :, 64:128],
                                      kTv[0:64, :, 64:128])
                nc.vector.tensor_copy(kblk3[64:128, :, 0:64],
                                      kTv[64:128, :, 0:64])
                nc.vector.tensor_copy(vblk1[0:64, :, 0:64], vTok[0:64, :, 0:64])
                nc.vector.tensor_copy(vblk1[64:128, :, 64:128],
                                      vTok[64:128, :, 64:128])
                nc.vector.tensor_copy(vblk2[0:64, :, 64:128],
                                      vTok[0:64, :, 64:128])
                nc.vector.tensor_copy(vblk2[64:128, :, 0:64],
                                      vTok[64:128, :, 0:64])

            # f2 params arrive during attention (DMA queues are idle then)
            nc.sync.dma_start(w2_t[:], w2_v[:, :, :])
            nc.sync.dma_start(b2p_s[:], b2p_in[:, :])
            nc.sync.dma_start(c2_s[:], c2_in[:, :])

            # ================= stage 2: attention per batch =================
            with (
                tc.tile_pool(name="ps_av", bufs=1, space="PSUM") as ps_av,
                tc.tile_pool(name="ps_s", bufs=2, space="PSUM") as ps_s,
                tc.tile_pool(name="attn", bufs=3) as attn,
            ):
                for bt in range(2):
                    t_b = bt * S
                    av = ps_av.tile([128, 2, 1024], F32, tag="av")
                    for qh in range(2):
                        q0 = t_b + qh * 1024
                        for kt in range(16):
                            c = bt * 16 + kt
                            for blk, vbt in ((kblk2, vblk1), (kblk3, vblk2)):
                                s = ps_s.tile([128, 1024], F32, tag="s")
                                for n2 in range(2):
                                    nc.tensor.matmul(
                                        s[:, n2 * 512:(n2 + 1) * 512],
                                        blk[:, c, :],
                                        qT[:, q0 + n2 * 512:q0 + (n2 + 1) * 512],
                                        start=True, stop=True)
                                a = attn.tile([128, 1024], BF16, tag="a")
                                nc.scalar.activation(a[:], s[:], SILU)
                                for n2 in range(2):
                                    nc.tensor.matmul(
                                        av[:, qh, n2 * 512:(n2 + 1) * 512],
                                        vbt[:, c, :],
                                        a[:, n2 * 512:(n2 + 1) * 512],
                                        start=(kt == 0 and blk is kblk2),
                                        stop=(kt == 15 and blk is kblk3))
                    # gate with u; ship the finished token range to DRAM
                    for qh in range(2):
                        g0 = t_b + qh * 1024
                        nc.vector.tensor_tensor(
                            gatedT[:, g0:g0 + 1024], av[:, qh, :],
                            uT[:, g0:g0 + 1024], MULT)
                        j0 = (bt * 2 + qh) * 2
                        nc.gpsimd.dma_start(a2a_in[j0],
                                            gatedT[:, g0:g0 + TO])
                        nc.gpsimd.dma_start(a2a_in[j0 + 1],
                                            gatedT[:, g0 + TO:g0 + 1024])

            # ================= stage 3: AllToAll + LN stats =================
            # LN is folded into f2: y = r*(G - mu*c2) + b2', where
            # G = g @ (gamma*W2), c2 = gamma@W2 (bc), b2' = beta@W2+b2 (bc).
            nc.gpsimd.collective_compute(
                "AllToAll", mybir.AluOpType.bypass,
                replica_groups=[[0, 1, 2, 3, 4, 5, 6, 7]],
                ins=[a2a_in[:]], outs=[a2a_out[:]])
            for j in range(KC):
                eng = nc.sync if j % 2 == 0 else nc.gpsimd
                eng.dma_start(gfull[:, j, :], a2a_out[j])

            rcol = sbp.tile([128, 4], F32)
            rmcol = sbp.tile([128, 4], F32)
            with (
                tc.tile_pool(name="ln", bufs=1) as ln,
                tc.tile_pool(name="ps_ln", bufs=1, space="PSUM") as ps_ln,
            ):
                st_sum = ps_ln.tile([1, TO], F32, tag="st_sum")
                st_sq = ps_ln.tile([1, TO], F32, tag="st_sq")
                for kc in range(KC):
                    nc.tensor.matmul(st_sum[:], ones_col[:], gfull[:, kc, :],
                                     start=(kc == 0), stop=(kc == KC - 1))
                for kc in range(KC):
                    sq = sbs.tile([128, TO], BF16, tag="sq")
                    nc.gpsimd.tensor_tensor(sq[:], gfull[:, kc, :],
                                            gfull[:, kc, :], MULT)
                    nc.tensor.matmul(st_sq[:], ones_col[:], sq[:],
                                     start=(kc == 0), stop=(kc == KC - 1))

                # move stats into a [128, 4] column layout (t = tt*128 + p)
                # so the whole scalar chain (incl. the 8-cyc/elem reciprocal)
                # runs across 128 lanes instead of one.
                mu_row = ln.tile([1, TO], F32, tag="mu_row")
                m2_row = ln.tile([1, TO], F32, tag="m2_row")
                nc.vector.tensor_copy(mu_row[:], st_sum[:])
                nc.vector.tensor_copy(m2_row[:], st_sq[:])
                # PE-transpose the stat rows into column layout [128, 4]
                stt = ps_ln.tile([128, 8], F32, tag="stt")
                for tt in range(4):
                    nc.tensor.transpose(stt[:, tt:tt + 1],
                                        mu_row[0:1, tt * 128:(tt + 1) * 128],
                                        one_f[0:1, 0:1])
                    nc.tensor.transpose(stt[:, 4 + tt:5 + tt],
                                        m2_row[0:1, tt * 128:(tt + 1) * 128],
                                        one_f[0:1, 0:1])
                mu_c = ln.tile([128, 4], F32, tag="mu_c")
                m2_c = ln.tile([128, 4], F32, tag="m2_c")
                nc.vector.tensor_copy(mu_c[:], stt[:, 0:4])
                nc.vector.tensor_copy(m2_c[:], stt[:, 4:8])

                varE = ln.tile([128, 4], F32, tag="varE")
                nc.vector.tensor_tensor(varE[:], mu_c[:], mu_c[:], MULT)
                nc.vector.tensor_tensor(varE[:], m2_c[:], varE[:], SUB)
                std = ln.tile([128, 4], F32, tag="std")
                nc.scalar.activation(std[:], varE[:], SQRT, bias=eps_t[:, 0:1],
                                     scale=1.0)
                r0 = ln.tile([128, 4], F32, tag="r0")
                nc.vector.reciprocal(r0[:], std[:])
                # one Newton step: r = r0 * (1.5 - 0.5 * (varE+eps) * r0^2)
                vpe = ln.tile([128, 4], F32, tag="vpe")
                nc.vector.tensor_scalar(vpe[:], varE[:], eps_t[:, 0:1], -0.5,
                                        ADD, MULT)
                nt1 = ln.tile([128, 4], F32, tag="nt1")
                nc.vector.tensor_tensor(nt1[:], r0[:], r0[:], MULT)
                nc.vector.tensor_tensor(nt1[:], nt1[:], vpe[:], MULT)
                nc.vector.tensor_scalar_add(nt1[:], nt1[:], 1.5)
                nc.vector.tensor_tensor(rcol[:], r0[:], nt1[:], MULT)
                nc.vector.tensor_tensor(rmcol[:], rcol[:], mu_c[:], MULT)

            # ================= stage 4: f2 + LN post-ops + store =================
            with (
                tc.tile_pool(name="ps_y", bufs=1, space="PSUM") as ps_y,
                tc.tile_pool(name="yout", bufs=2) as yout,
            ):
                for tt in range(4):
                    psy = ps_y.tile([128, D], F32, tag=f"y{tt}", name=f"psy{tt}")
                    for kc in range(KC):
                        for n2 in range(2):
                            nc.tensor.matmul(
                                psy[:, n2 * 512:(n2 + 1) * 512],
                                gfull[:, kc, tt * 128:(tt + 1) * 128],
                                w2_t[:, kc, n2 * 512:(n2 + 1) * 512],
                                start=(kc == 0), stop=(kc == KC - 1))
                    # y = psy * r - (c2 * r*mu - b2')
                    mterm = yout.tile([128, D], F32, tag="mt")
                    nc.gpsimd.tensor_scalar(mterm[:], c2_s[:],
                                            rmcol[:, tt:tt + 1], 0.0, MULT, ADD)
                    nc.gpsimd.tensor_tensor(mterm[:], mterm[:], b2p_s[:], SUB)
                    yo = yout.tile([128, D], F32, tag="yo")
                    nc.vector.tensor_scalar(yo[:], psy[:],
                                            rcol[:, tt:tt + 1], 0.0, MULT, ADD)
                    nc.vector.tensor_tensor(yo[:], yo[:], mterm[:], SUB)
                    eng = nc.sync if tt % 2 == 0 else nc.gpsimd
                    eng.dma_start(y_s[tt * 128:(tt + 1) * 128, :], yo[:])

    nc.compile()
    return nc


def _get_nc():
    if "nc" not in _CACHE:
        _CACHE["nc"] = _build()
    return _CACHE["nc"]


def kernel(x, W1, b1, W2, b2, gamma, beta, **kw):
    nc = _get_nc()
    bf16 = ml_dtypes.bfloat16
    x = np.asarray(x, dtype=np.float32)
    W1 = np.asarray(W1, dtype=np.float32)
    b1 = np.asarray(b1, dtype=np.float32)
    W2f = np.asarray(W2, dtype=np.float32)
    gamma = np.asarray(gamma, dtype=np.float32)
    beta = np.asarray(beta, dtype=np.float32)
    b2 = np.asarray(b2, dtype=np.float32)
    # LN folded into f2: G = g @ (gamma*W2); y = r*G - (r*mu)*c2 + b2p
    W2bf = np.ascontiguousarray((W2f * gamma[:, None]).astype(bf16))
    c2 = gamma @ W2f
    b2p = beta @ W2f + b2
    c2_bc = np.ascontiguousarray(np.broadcast_to(c2, (128, D)), dtype=np.float32)
    b2p_bc = np.ascontiguousarray(np.broadcast_to(b2p, (128, D)), dtype=np.float32)
    # [D, 4096]: both batches concatenated along tokens
    xTall = np.ascontiguousarray(
        x.transpose(2, 0, 1).reshape(D, TT).astype(bf16))

    in_maps = []
    u0, v0, q0, k0 = 0, D, 2 * D, 3 * D
    for c in range(8):
        cs = 128 * c
        in_maps.append({
            "xT": xTall,
            "w1q": np.ascontiguousarray(W1[:, q0 + cs:q0 + cs + 128].astype(bf16)),
            "w1k": np.ascontiguousarray(W1[:, k0 + cs:k0 + cs + 128].astype(bf16)),
            "w1v": np.ascontiguousarray(W1[:, v0 + cs:v0 + cs + 128].astype(bf16)),
            "w1u": np.ascontiguousarray(W1[:, u0 + cs:u0 + cs + 128].astype(bf16)),
            "b1q": np.ascontiguousarray(b1[q0 + cs:q0 + cs + 128].reshape(128, 1)),
            "b1k": np.ascontiguousarray(b1[k0 + cs:k0 + cs + 128].reshape(128, 1)),
            "b1v": np.ascontiguousarray(b1[v0 + cs:v0 + cs + 128].reshape(128, 1)),
            "b1u": np.ascontiguousarray(b1[u0 + cs:u0 + cs + 128].reshape(128, 1)),
            "w2": W2bf,
            "b2p_bc": b2p_bc,
            "c2_bc": c2_bc,
        })
    res = run_bass_kernel_spmd(nc, in_maps, core_ids=list(range(8)), **kw)
    y = np.empty((B, S, D), dtype=np.float32)
    for c in range(8):
        b = c // 4
        t0 = 512 * (c % 4)
        y[b, t0:t0 + 512, :] = res.results[c]["y_s"]
    if kw:
        _CACHE["last_res"] = res
    return y


# revision 34
# speedup vs baseline: 1.0513x; 1.0513x over previous
"""HSTU block kernel for 8 Trainium2 NeuronCores — head-sharded attention.

Sharding: core c owns heads {2c, 2c+1} (a 128-feature slice of each of the
q/k/v/u projections) for ALL 4096 tokens (both batches). f1 computes only
those W1 columns, so attention is fully local per core — no kv exchange.
After gating (av*u, feature-major) one 8-core AllToAll (1 MB) re-shards
features -> tokens; LN + f2 then run token-parallel on 512 tokens per core.

All matmuls run in bf16 (full PE rate, half the DMA/SBUF of fp32). The host
pre-transposes x and pre-slices W1/b1 so the device does no input transposes
and only ~20 fat DMAs. Scores use K=64 contraction with the two heads of a
pair packed on partitions via tile_position; AV accumulates both heads into
one PSUM tile via output tile_position. The silu(scores)/S scaling is folded
into LayerNorm via eps' = S^2 * eps (LN is scale-invariant except for eps).
"""

import sys

sys.path.insert(0, "/opt/trn_rl_repo")

import ml_dtypes
import numpy as np

import concourse.bass as bass
import concourse.mybir as mybir
import concourse.tile as tile
from concourse import bacc
from concourse.bass_utils import run_bass_kernel_spmd
from concourse.masks import make_identity

F32 = mybir.dt.float32
F32R = mybir.dt.float32r
BF16 = mybir.dt.bfloat16
SILU = mybir.ActivationFunctionType.Silu
SQRT = mybir.ActivationFunctionType.Sqrt
MULT = mybir.AluOpType.mult
ADD = mybir.AluOpType.add
SUB = mybir.AluOpType.subtract

B, S, D = 2, 2048, 1024
TT = B * S         # 4096 tokens total (f1/attention scope per core)
TO = 512           # output tokens per core
KC = D // 128      # 8 feature chunks of the model dim
NTC = TT // 128    # 32 token chunks
EPS_EFF = float(S) * float(S) * 1e-5

_CACHE = {}


def _build():
    nc = bacc.Bacc(None, target_bir_lowering=False, num_devices=8)

    xT = nc.dram_tensor("xT", [D, TT], BF16, kind="ExternalInput")
    w1q = nc.dram_tensor("w1q", [D, 128], BF16, kind="ExternalInput")
    w1k = nc.dram_tensor("w1k", [D, 128], BF16, kind="ExternalInput")
    w1v = nc.dram_tensor("w1v", [D, 128], BF16, kind="ExternalInput")
    w1u = nc.dram_tensor("w1u", [D, 128], BF16, kind="ExternalInput")
    b1q = nc.dram_tensor("b1q", [128, 1], F32, kind="ExternalInput")
    b1k = nc.dram_tensor("b1k", [128, 1], F32, kind="ExternalInput")
    b1v = nc.dram_tensor("b1v", [128, 1], F32, kind="ExternalInput")
    b1u = nc.dram_tensor("b1u", [128, 1], F32, kind="ExternalInput")
    w2 = nc.dram_tensor("w2", [D, D], BF16, kind="ExternalInput")   # gamma-scaled
    b2p_in = nc.dram_tensor("b2p_bc", [128, D], F32, kind="ExternalInput")
    c2_in = nc.dram_tensor("c2_bc", [128, D], F32, kind="ExternalInput")
    y_s = nc.dram_tensor("y_s", [TO, D], F32, kind="ExternalOutput")

    xT_v = xT.rearrange("(kc p) t -> p kc t", p=128)
    w1q_v = w1q.rearrange("(kc p) f -> p kc f", p=128)
    w1k_v = w1k.rearrange("(kc p) f -> p kc f", p=128)
    w1v_v = w1v.rearrange("(kc p) f -> p kc f", p=128)
    w1u_v = w1u.rearrange("(kc p) f -> p kc f", p=128)
    w2_v = w2.rearrange("(kc p) n -> p kc n", p=128)

    with tile.TileContext(nc) as tc:
        with (
            tc.tile_pool(name="persist", bufs=1) as sbp,
            tc.tile_pool(name="small", bufs=2) as sbs,
            tc.tile_pool(name="dram", bufs=1, space="DRAM") as dram,
        ):
            # ---- constants / small params
            ident = sbp.tile([128, 128], BF16)
            make_identity(nc, ident[:])
            # stats lhsT pre-scaled by 1/D so the matmul yields means directly
            ones_col = sbp.tile([128, 1], BF16)
            nc.vector.memset(ones_col[:], 1.0 / D)
            one_f = sbp.tile([1, 1], F32)
            nc.vector.memset(one_f[:], 1.0)

            b1q_s = sbp.tile([128, 1], F32)
            b1k_s = sbp.tile([128, 1], F32)
            b1v_s = sbp.tile([128, 1], F32)
            b1u_s = sbp.tile([128, 1], F32)
            b2p_s = sbp.tile([128, D], F32)
            c2_s = sbp.tile([128, D], F32)
            eps_t = sbp.tile([128, 1], F32)
            nc.vector.memset(eps_t[:], EPS_EFF)
            nc.sync.dma_start(b1q_s[:], b1q[:, :])
            nc.sync.dma_start(b1k_s[:], b1k[:, :])
            nc.sync.dma_start(b1v_s[:], b1v[:, :])
            nc.sync.dma_start(b1u_s[:], b1u[:, :])

            # ---- persistent activations (bf16)
            kT = sbp.tile([128, TT], BF16)
            qT = sbp.tile([128, TT], BF16)
            uT = sbp.tile([128, TT], BF16)
            vTok = sbp.tile([128, NTC, 128], BF16)   # v token-major
            gatedT = sbp.tile([128, TT], BF16)
            gfull = sbp.tile([128, KC, TO], BF16)

            # block-diagonal head-pair packs: every score/AV matmul gets a
            # full 128x128 stationary tile (64-wide tiles run at half rate).
            # kblk2 block c: cols 0:64 = head-A k for kv [128c,+64) on rows
            # 0:64; cols 64:128 = head-B k for kv [128c+64,+128) on rows
            # 64:128. kblk3 is the mirror (B first half / A second half).
            kblk2 = sbp.tile([128, NTC, 128], BF16)
            kblk3 = sbp.tile([128, NTC, 128], BF16)
            vblk1 = sbp.tile([128, NTC, 128], BF16)
            vblk2 = sbp.tile([128, NTC, 128], BF16)

            w1q_t = sbp.tile([128, KC, 128], BF16)
            w1k_t = sbp.tile([128, KC, 128], BF16)
            w1v_t = sbp.tile([128, KC, 128], BF16)
            w1u_t = sbp.tile([128, KC, 128], BF16)
            w2_t = sbp.tile([128, KC, D], BF16)

            a2a_in = dram.tile([8, 128, TO], BF16, name="a2a_in")
            a2a_out = dram.tile([8, 128, TO], BF16, name="a2a_out")

            # ================= stage 1: f1 (k, q, v, u) =================
            with (
                tc.tile_pool(name="xpool", bufs=1) as xp,
                tc.tile_pool(name="ps_f1", bufs=2, space="PSUM") as ps_f1,
                tc.tile_pool(name="ps_tr", bufs=4, space="PSUM") as ps_tr,
            ):
                xTs = xp.tile([128, KC, TT], BF16)   # 64 KB/partition
                # weights interleaved with fine-grained x slices so the first
                # k-projection matmuls start as early as possible
                nc.sync.dma_start(w1k_t[:], w1k_v[:, :, :])
                nc.sync.dma_start(xTs[:, :, 0:512], xT_v[:, :, 0:512])
                nc.sync.dma_start(xTs[:, :, 512:1024], xT_v[:, :, 512:1024])
                nc.sync.dma_start(w1q_t[:], w1q_v[:, :, :])
                nc.sync.dma_start(w1v_t[:], w1v_v[:, :, :])
                nc.sync.dma_start(w1u_t[:], w1u_v[:, :, :])
                for h8 in range(2, 8):
                    nc.sync.dma_start(xTs[:, :, h8 * 512:(h8 + 1) * 512],
                                      xT_v[:, :, h8 * 512:(h8 + 1) * 512])

                nc.vector.memset(kblk2[:], 0.0)
                nc.vector.memset(kblk3[:], 0.0)
                nc.vector.memset(vblk1[:], 0.0)
                nc.vector.memset(vblk2[:], 0.0)
                # feature-major projections: out [128 f, 1024 t] per quarter
                for wt, bt, dst in ((w1k_t, b1k_s, kT), (w1q_t, b1q_s, qT),
                                    (w1v_t, b1v_s, None), (w1u_t, b1u_s, uT)):
                    for qh in range(4):
                        ps = ps_f1.tile([128, 1024], F32, tag="f1")
                        for kc in range(KC):
                            for n2 in range(2):
                                nc.tensor.matmul(
                                    ps[:, n2 * 512:(n2 + 1) * 512], wt[:, kc, :],
                                    xTs[:, kc, qh * 1024 + n2 * 512:qh * 1024 + (n2 + 1) * 512],
                                    start=(kc == 0), stop=(kc == KC - 1))
                        if dst is not None:
                            nc.scalar.activation(
                                dst[:, qh * 1024:(qh + 1) * 1024], ps[:],
                                SILU, bias=bt[:, 0:1], scale=1.0)
                        else:
                            # v: silu into a staging tile, then PE-transpose
                            vf = sbs.tile([128, 1024], BF16, tag="vf")
                            nc.scalar.activation(vf[:], ps[:], SILU,
                                                 bias=bt[:, 0:1], scale=1.0)
                            for t8 in range(8):
                                pt = ps_tr.tile([128, 128], BF16, tag="tr")
                                nc.tensor.transpose(
                                    pt[:], vf[:, t8 * 128:(t8 + 1) * 128],
                                    ident[:])
                                nc.vector.tensor_copy(
                                    vTok[:, qh * 8 + t8, :], pt[:])

                # build the block-diagonal packs (strided bulk copies)
                kTv = kT[:].rearrange("p (c t) -> p c t", t=128)
                nc.vector.tensor_copy(kblk2[0:64, :, 0:64], kTv[0:64, :, 0:64])
                nc.vector.tensor_copy(kblk2[64:128, :, 64:128],
                                      kTv[64:128, :, 64:128])
                nc.vector.tensor_copy(kblk3[0:64, :, 64:128],
                                      kTv[0:64, :, 64:128])
                nc.vector.tensor_copy(kblk3[64:128, :, 0:64],
                                      kTv[64:128, :, 0:64])
                nc.vector.tensor_copy(vblk1[0:64, :, 0:64], vTok[0:64, :, 0:64])
                nc.vector.tensor_copy(vblk1[64:128, :, 64:128],
                                      vTok[64:128, :, 64:128])
                nc.vector.tensor_copy(vblk2[0:64, :, 64:128],
                                      vTok[0:64, :, 64:128])
                nc.vector.tensor_copy(vblk2[64:128, :, 0:64],
                                      vTok[64:128, :, 0:64])

            # f2 params arrive during attention (DMA queues are idle then)
            nc.sync.dma_start(w2_t[:], w2_v[:, :, :])
            nc.sync.dma_start(b2p_s[:], b2p_in[:, :])
            nc.sync.dma_start(c2_s[:], c2_in[:, :])

            # ================= stage 2: attention per batch =================
            with (
                tc.tile_pool(name="ps_av", bufs=1, space="PSUM") as ps_av,
                tc.tile_pool(name="ps_s", bufs=2, space="PSUM") as ps_s,
                tc.tile_pool(name="attn", bufs=3) as attn,
            ):
                for bt in range(2):
                    t_b = bt * S
                    av = ps_av.tile([128, 2, 1024], F32, tag="av")
                    for qh in range(2):
                        q0 = t_b + qh * 1024
                        for kt in range(16):
                            c = bt * 16 + kt
                            for blk, vbt in ((kblk2, vblk1), (kblk3, vblk2)):
                                s = ps_s.tile([128, 1024], F32, tag="s")
                                for n2 in range(2):
                                    nc.tensor.matmul(
                                        s[:, n2 * 512:(n2 + 1) * 512],
                                        blk[:, c, :],
                                        qT[:, q0 + n2 * 512:q0 + (n2 + 1) * 512],
                                        start=True, stop=True)
                                a = attn.tile([128, 1024], BF16, tag="a")
                                nc.scalar.activation(a[:], s[:], SILU)
                                for n2 in range(2):
                                    nc.tensor.matmul(
                                        av[:, qh, n2 * 512:(n2 + 1) * 512],
                                        vbt[:, c, :],
                                        a[:, n2 * 512:(n2 + 1) * 512],
                                        start=(kt == 0 and blk is kblk2),
                                        stop=(kt == 15 and blk is kblk3))
                    # gate with u; ship the finished token range to DRAM
                    for qh in range(2):
                        g0 = t_b + qh * 1024
                        nc.vector.tensor_tensor(
                            gatedT[:, g0:g0 + 1024], av[:, qh, :],
                            uT[:, g0:g0 + 1024], MULT)
                        j0 = (bt * 2 + qh) * 2
                        nc.gpsimd.dma_start(a2a_in[j0],
                                            gatedT[:, g0:g0 + TO])
                        nc.gpsimd.dma_start(a2a_in[j0 + 1],
                                            gatedT[:, g0 + TO:g0 + 1024])

            # ================= stage 3: AllToAll + LN stats =================
            # LN is folded into f2: y = r*(G - mu*c2) + b2', where
            # G = g @ (gamma*W2), c2 = gamma@W2 (bc), b2' = beta@W2+b2 (bc).
            nc.gpsimd.collective_compute(
                "AllToAll", mybir.AluOpType.bypass,
                replica_groups=[[0, 1, 2, 3, 4, 5, 6, 7]],
                ins=[a2a_in[:]], outs=[a2a_out[:]])
            for j in range(KC):
                eng = nc.sync if j % 2 == 0 else nc.gpsimd
                eng.dma_start(gfull[:, j, :], a2a_out[j])

            rcol = sbp.tile([128, 4], F32)
            rmcol = sbp.tile([128, 4], F32)
            with (
                tc.tile_pool(name="ln", bufs=1) as ln,
                tc.tile_pool(name="ps_ln", bufs=1, space="PSUM") as ps_ln,
            ):
                st_sum = ps_ln.tile([1, TO], F32, tag="st_sum")
                st_sq = ps_ln.tile([1, TO], F32, tag="st_sq")
                for kc in range(KC):
                    nc.tensor.matmul(st_sum[:], ones_col[:], gfull[:, kc, :],
                                     start=(kc == 0), stop=(kc == KC - 1))
                for kc in range(KC):
                    sq = sbs.tile([128, TO], BF16, tag="sq")
                    nc.vector.tensor_tensor(sq[:], gfull[:, kc, :],
                                            gfull[:, kc, :], MULT)
                    nc.tensor.matmul(st_sq[:], ones_col[:], sq[:],
                                     start=(kc == 0), stop=(kc == KC - 1))

                # move stats into a [128, 4] column layout (t = tt*128 + p)
                # so the whole scalar chain (incl. the 8-cyc/elem reciprocal)
                # runs across 128 lanes instead of one.
                mu_row = ln.tile([1, TO], F32, tag="mu_row")
                m2_row = ln.tile([1, TO], F32, tag="m2_row")
                nc.vector.tensor_copy(mu_row[:], st_sum[:])
                nc.vector.tensor_copy(m2_row[:], st_sq[:])
                # PE-transpose the stat rows into column layout [128, 4]
                stt = ps_ln.tile([128, 8], F32, tag="stt")
                for tt in range(4):
                    nc.tensor.transpose(stt[:, tt:tt + 1],
                                        mu_row[0:1, tt * 128:(tt + 1) * 128],
                                        one_f[0:1, 0:1])
                    nc.tensor.transpose(stt[:, 4 + tt:5 + tt],
                                        m2_row[0:1, tt * 128:(tt + 1) * 128],
                                        one_f[0:1, 0:1])
                mu_c = ln.tile([128, 4], F32, tag="mu_c")
                m2_c = ln.tile([128, 4], F32, tag="m2_c")
                nc.vector.tensor_copy(mu_c[:], stt[:, 0:4])
                nc.vector.tensor_copy(m2_c[:], stt[:, 4:8])

                varE = ln.tile([128, 4], F32, tag="varE")
                nc.vector.tensor_tensor(varE[:], mu_c[:], mu_c[:], MULT)
                nc.vector.tensor_tensor(varE[:], m2_c[:], varE[:], SUB)
                std = ln.tile([128, 4], F32, tag="std")
                nc.scalar.activation(std[:], varE[:], SQRT, bias=eps_t[:, 0:1],
                                     scale=1.0)
                r0 = ln.tile([128, 4], F32, tag="r0")
                nc.vector.reciprocal(r0[:], std[:])
                # one Newton step: r = r0 * (1.5 - 0.5 * (varE+eps) * r0^2)
                vpe = ln.tile([128, 4], F32, tag="vpe")
                nc.vector.tensor_scalar(vpe[:], varE[:], eps_t[:, 0:1], -0.5,
                                        ADD, MULT)
                nt1 = ln.tile([128, 4], F32, tag="nt1")
                nc.vector.tensor_tensor(nt1[:], r0[:], r0[:], MULT)
                nc.vector.tensor_tensor(nt1[:], nt1[:], vpe[:], MULT)
                nc.vector.tensor_scalar_add(nt1[:], nt1[:], 1.5)
                nc.vector.tensor_tensor(rcol[:], r0[:], nt1[:], MULT)
                nc.vector.tensor_tensor(rmcol[:], rcol[:], mu_c[:], MULT)

            # ================= stage 4: f2 + LN post-ops + store =================
            with (
                tc.tile_pool(name="ps_y", bufs=1, space="PSUM") as ps_y,
                tc.tile_pool(name="yout", bufs=2) as yout,
            ):
                for tt in range(4):
                    psy = ps_y.tile([128, D], F32, tag=f"y{tt}", name=f"psy{tt}")
                    for kc in range(KC):
                        for n2 in range(2):
                            nc.tensor.matmul(
                                psy[:, n2 * 512:(n2 + 1) * 512],
                                gfull[:, kc, tt * 128:(tt + 1) * 128],
                                w2_t[:, kc, n2 * 512:(n2 + 1) * 512],
                                start=(kc == 0), stop=(kc == KC - 1))
                    # y = psy * r - (c2 * r*mu - b2')
                    mterm = yout.tile([128, D], F32, tag="mt")
                    nc.vector.tensor_scalar(mterm[:], c2_s[:],
                                            rmcol[:, tt:tt + 1], 0.0, MULT, ADD)
                    nc.vector.tensor_tensor(mterm[:], mterm[:], b2p_s[:], SUB)
                    yo = yout.tile([128, D], F32, tag="yo")
                    nc.vector.tensor_scalar(yo[:], psy[:],
                                            rcol[:, tt:tt + 1], 0.0, MULT, ADD)
                    nc.vector.tensor_tensor(yo[:], yo[:], mterm[:], SUB)
                    eng = nc.sync if tt % 2 == 0 else nc.gpsimd
                    eng.dma_start(y_s[tt * 128:(tt + 1) * 128, :], yo[:])

    nc.compile()
    return nc


def _get_nc():
    if "nc" not in _CACHE:
        _CACHE["nc"] = _build()
    return _CACHE["nc"]


def kernel(x, W1, b1, W2, b2, gamma, beta, **kw):
    nc = _get_nc()
    bf16 = ml_dtypes.bfloat16
    x = np.asarray(x, dtype=np.float32)
    W1 = np.asarray(W1, dtype=np.float32)
    b1 = np.asarray(b1, dtype=np.float32)
    W2f = np.asarray(W2, dtype=np.float32)
    gamma = np.asarray(gamma, dtype=np.float32)
    beta = np.asarray(beta, dtype=np.float32)
    b2 = np.asarray(b2, dtype=np.float32)
    # LN folded into f2: G = g @ (gamma*W2); y = r*G - (r*mu)*c2 + b2p
    W2bf = np.ascontiguousarray((W2f * gamma[:, None]).astype(bf16))
    c2 = gamma @ W2f
    b2p = beta @ W2f + b2
    c2_bc = np.ascontiguousarray(np.broadcast_to(c2, (128, D)), dtype=np.float32)
    b2p_bc = np.ascontiguousarray(np.broadcast_to(b2p, (128, D)), dtype=np.float32)
    # [D, 4096]: both batches concatenated along tokens
    xTall = np.ascontiguousarray(
        x.transpose(2, 0, 1).reshape(D, TT).astype(bf16))

    in_maps = []
    u0, v0, q0, k0 = 0, D, 2 * D, 3 * D
    for c in range(8):
        cs = 128 * c
        in_maps.append({
            "xT": xTall,
            "w1q": np.ascontiguousarray(W1[:, q0 + cs:q0 + cs + 128].astype(bf16)),
            "w1k": np.ascontiguousarray(W1[:, k0 + cs:k0 + cs + 128].astype(bf16)),
            "w1v": np.ascontiguousarray(W1[:, v0 + cs:v0 + cs + 128].astype(bf16)),
            "w1u": np.ascontiguousarray(W1[:, u0 + cs:u0 + cs + 128].astype(bf16)),
            "b1q": np.ascontiguousarray(b1[q0 + cs:q0 + cs + 128].reshape(128, 1)),
            "b1k": np.ascontiguousarray(b1[k0 + cs:k0 + cs + 128].reshape(128, 1)),
            "b1v": np.ascontiguousarray(b1[v0 + cs:v0 + cs + 128].reshape(128, 1)),
            "b1u": np.ascontiguousarray(b1[u0 + cs:u0 + cs + 128].reshape(128, 1)),
            "w2": W2bf,
            "b2p_bc": b2p_bc,
            "c2_bc": c2_bc,
        })
    res = run_bass_kernel_spmd(nc, in_maps, core_ids=list(range(8)), **kw)
    y = np.empty((B, S, D), dtype=np.float32)
    for c in range(8):
        b = c // 4
        t0 = 512 * (c % 4)
        y[b, t0:t0 + 512, :] = res.results[c]["y_s"]
    if kw:
        _CACHE["last_res"] = res
    return y
